# revision 10
# baseline (speedup 1.0000x reference)
"""Trainium2 Bass kernel for CAGKE (Gaussian-kernel spike embedding), v9a.

Math: psedu[t] = sum_d softmax(weight)[d] * (spikes (*) K_d)[t] + noise[t],
then global min-max normalization. The softmax-weighted kernel-bank sum
commutes with the convolution (linearity), so psedu = spikes (*) kbar +
noise with kbar(delta) = sum_d sw_d*(C/sigma_d)*exp(-delta^2/(2 s_d^2)),
live taps |delta| <= 44 in f32.

Host-side input prep (O(T) / O(D*taps), ~0.1% of module FLOPs): the
D=128 kernel bank is folded into the banded-Toeplitz operand m_all; the
spike train is thresholded and laid out as the shifted transposed grid
spT[qt, c'] = (X[128c'-45+qt] > 0.5); the softmax denominator (min-max
norm is invariant under positive affine maps) is folded into the noise
as noise2 = (sum_d exp(w_d)/C) * noise.

The device runs the convolution - 99.9% of the work - as two
128-contraction PE matmuls over the shifted spike grid:

  psedu[128c+p] = sum_qt spT[qt, c]  * m_all[qt, 128+p]
               +  sum_qt spT[qt, c+1]* m_all[qt, p]

Global min/max go through a PE transpose + mconst broadcast matmul.
Normalize + store runs in row halves on both HWDGE rings.

All 8 cores run the identical replicated program (total I/O ~160KB;
collectives would cost more than they save); host takes core 0's output.
"""

import os
import sys

for _p in ("/opt/trn_rl_repo", "/root/.axon_site/_ro/trn_rl_repo"):
    if os.path.isdir(_p) and _p not in sys.path:
        sys.path.insert(0, _p)

import numpy as np

T = 8192  # in_length
D = 128  # embed_dim (kernel bank size)
NCORES = 8
COLS = T // 128  # 64 output blocks of 128 time steps
SCOLS = COLS + 1  # 65 shifted spike blocks
SHIFT = 45  # spike grid shift: block c' covers X[128c'-45 : 128c'+83)
GAUSS_C = 0.39894228  # 1/sqrt(2*pi) as hardcoded in the source module

_CACHE = {}


def _build_bass():
    import concourse.bass as bass
    import concourse.tile as tile
    from concourse import bacc, mybir

    f32 = mybir.dt.float32
    bf16 = mybir.dt.bfloat16
    nc = bacc.Bacc("TRN2", target_bir_lowering=False, debug=False, num_devices=NCORES)

    sp_d = nc.dram_tensor("spT", [128, SCOLS], bf16, kind="ExternalInput")
    m_d = nc.dram_tensor("m_all", [128, 256], bf16, kind="ExternalInput")
    n_d = nc.dram_tensor("noise2", [COLS, 128], f32, kind="ExternalInput")
    o_d = nc.dram_tensor("out", [COLS, 128], f32, kind="ExternalOutput")

    with tile.TileContext(nc) as tc:
        with (
            tc.tile_pool(name="sb", bufs=1) as sb,
            tc.tile_pool(name="ps", bufs=1, space="PSUM") as ps,
        ):
            # ---- input DMAs: spike grid halves split across both HWDGE
            # rings, kernel table on sync, noise on scalar ----
            m_all = sb.tile([128, 256], bf16, tag="L")
            nc.sync.dma_start(out=m_all[:], in_=m_d.ap())
            spT = sb.tile([128, SCOLS], bf16)
            nrm = sb.tile([COLS, 128], f32)
            nc.scalar.dma_start(out=nrm[:], in_=n_d.ap())
            nc.sync.dma_start(
                out=spT[0:64, :],
                in_=bass.AP(tensor=sp_d.ap().tensor, offset=0, ap=[[SCOLS, 64], [1, SCOLS]]),
            )
            nc.scalar.dma_start(
                out=spT[64:128, :],
                in_=bass.AP(
                    tensor=sp_d.ap().tensor, offset=64 * SCOLS, ap=[[SCOLS, 64], [1, SCOLS]]
                ),
            )

            # ---- constants ----
            id65 = sb.tile([SCOLS, SCOLS], f32)  # identity (pk transpose)
            nc.gpsimd.memset(id65[:], 0.0)
            nc.gpsimd.affine_select(
                out=id65[:], in_=id65[:], compare_op=mybir.AluOpType.not_equal,
                fill=1.0, base=0, pattern=[[-1, SCOLS]], channel_multiplier=1,
            )
            ones2 = sb.tile([2, COLS], bf16)  # stat broadcast weights
            nc.gpsimd.memset(ones2[:], 1.0)
            # mconst = [[1, 0], [1, -1]]: maps g = [gmax, -gmin] to
            # stat = [range, gmin] via one broadcast matmul
            mconst = sb.tile([2, 2], bf16)
            nc.gpsimd.memset(mconst[:], 1.0)
            nc.gpsimd.affine_select(
                out=mconst[:], in_=mconst[:], compare_op=mybir.AluOpType.not_equal,
                fill=0.0, base=-1, pattern=[[1, 2]], channel_multiplier=2,
            )
            nc.gpsimd.affine_select(
                out=mconst[:], in_=mconst[:], compare_op=mybir.AluOpType.not_equal,
                fill=-1.0, base=-3, pattern=[[1, 2]], channel_multiplier=2,
            )

            # ---- banded conv: psedu_rm[c, p] = sum_qt spT[qt, c+b] M_b[qt, p] ----
            conv_ps = ps.tile([COLS, 128], f32, tag="ps_c")
            nc.tensor.matmul(
                conv_ps[:], lhsT=spT[:, 0:COLS], rhs=m_all[:, 128:256],
                start=True, stop=False,
            )
            nc.tensor.matmul(
                conv_ps[:], lhsT=spT[:, 1:SCOLS], rhs=m_all[:, 0:128],
                start=False, stop=True,
            )

            # ---- noise add, then global min-max normalize ----
            ps_rm = sb.tile([COLS, 128], f32)
            nc.vector.tensor_tensor(
                out=ps_rm[:], in0=conv_ps[:], in1=nrm[:], op=mybir.AluOpType.add,
            )
            pk = sb.tile([COLS, 2], f32)  # per-partition [max, -min]
            nc.vector.tensor_reduce(
                out=pk[:, 0:1], in_=ps_rm[:], axis=mybir.AxisListType.X,
                op=mybir.AluOpType.max,
            )
            nc.vector.tensor_reduce(
                out=pk[:, 1:2], in_=ps_rm[:], axis=mybir.AxisListType.X,
                op=mybir.AluOpType.min, negate=True,
            )
            pk_ps = ps.tile([2, COLS], f32, tag="ps_k")
            nc.tensor.transpose(pk_ps[:], pk[:], id65[0:COLS, 0:COLS])
            g = sb.tile([2, 1], f32)
            nc.vector.tensor_reduce(
                out=g[:], in_=pk_ps[:], axis=mybir.AxisListType.X,
                op=mybir.AluOpType.max,
            )  # g = [gmax, -gmin]
            gg = sb.tile([2, 2], bf16)
            nc.vector.tensor_scalar_mul(gg[:], in0=mconst[:], scalar1=g[:, 0:1])
            stat_ps = ps.tile([COLS, 2], f32, tag="ps_s")
            nc.tensor.matmul(
                stat_ps[:], lhsT=ones2[:], rhs=gg[:], start=True, stop=True
            )  # [range, gmin] on all 64 partitions
            inv_rng = sb.tile([COLS, 1], f32)
            nc.vector.reciprocal(inv_rng[:], stat_ps[:, 0:1])
            # normalize + store in row halves on both rings so the second
            # half's compute overlaps the first half's store
            outt = sb.tile([COLS, 128], f32)
            nc.vector.tensor_scalar(
                out=outt[0:32, :], in0=ps_rm[0:32, :], scalar1=stat_ps[0:32, 1:2],
                scalar2=inv_rng[0:32, 0:1], op0=mybir.AluOpType.subtract,
                op1=mybir.AluOpType.mult,
            )  # (x - gmin) / range
            nc.sync.dma_start(out=o_d.ap()[0:32, :], in_=outt[0:32, :])
            nc.vector.tensor_scalar(
                out=outt[32:COLS, :], in0=ps_rm[32:COLS, :],
                scalar1=stat_ps[32:COLS, 1:2], scalar2=inv_rng[32:COLS, 0:1],
                op0=mybir.AluOpType.subtract, op1=mybir.AluOpType.mult,
            )
            nc.scalar.dma_start(out=o_d.ap()[32:COLS, :], in_=outt[32:COLS, :])

    nc.compile()
    return nc


def _get_nc():
    if "nc" not in _CACHE:
        _CACHE["nc"] = _build_bass()
    return _CACHE["nc"]


def _run(in_map, trace=False, **kwargs):
    from concourse.bass_utils import run_bass_kernel_spmd

    nc = _get_nc()
    return run_bass_kernel_spmd(
        nc, [in_map] * NCORES, core_ids=list(range(NCORES)), trace=trace, **kwargs
    )


def _prepare(X, weight, noise, sigma):
    """Host-side input prep: fold the kernel bank into the banded-Toeplitz
    conv operand (linearity of the softmax-weighted sum; min-max norm is
    invariant to the overall softmax scale, which lands on the noise),
    threshold + shift-transpose the spike train."""
    import ml_dtypes

    X = np.ascontiguousarray(X, dtype=np.float32).reshape(T)
    weight = np.asarray(weight, dtype=np.float64).reshape(D)
    noise = np.ascontiguousarray(noise, dtype=np.float32).reshape(COLS, 128)
    sigma = np.asarray(sigma, dtype=np.float64).reshape(D)

    # kbar'(delta) = sum_d exp(w_d)/sigma_d * exp(-delta^2/(2 sigma_d^2));
    # psedu' = kbar' (*) spikes + (esum/C)*noise is a positive affine image
    # of the reference psedu, so the min-max normalized output matches.
    delta = np.arange(-256, 256, dtype=np.float64)
    kb = (
        (np.exp(weight) / sigma)[:, None]
        * np.exp(-(delta[None, :] ** 2) / (2.0 * sigma * sigma)[:, None])
    ).sum(0)  # kb[j] = kbar'(j - 256)

    # m_all[qt, p + 128*(1-b)] = kbar'(p + 44 - qt - 128b), b in {0, 1}
    qt = np.arange(128)[:, None]
    p = np.arange(128)[None, :]
    m0 = kb[256 + p + 44 - qt]  # b=0 -> cols 128:256
    m1 = kb[256 + p + 44 - qt - 128]  # b=1 -> cols 0:128
    m_all = np.concatenate([m1, m0], axis=1)

    # spT[qt, c'] = spikes[128c' - 45 + qt], zero outside [0, T)
    spikes = (X > 0.5).astype(np.float32)
    c = np.arange(SCOLS)[None, :]
    idx = 128 * c - SHIFT + np.arange(128)[:, None]
    valid = (idx >= 0) & (idx < T)
    spT = np.where(valid, spikes[np.clip(idx, 0, T - 1)], 0.0)

    es = np.exp(weight).sum() / GAUSS_C
    noise2 = (es * noise.astype(np.float64)).astype(np.float32)

    return {
        "spT": spT.astype(ml_dtypes.bfloat16),
        "m_all": m_all.astype(ml_dtypes.bfloat16),
        "noise2": noise2,
    }


def kernel(X, weight, noise, sigma):
    in_map = _prepare(X, weight, noise, sigma)
    res = _run(in_map).results
    return res[0]["out"].reshape(1, T)
